# revision 4
# baseline (speedup 1.0000x reference)
"""Two-layer GraphSAGE-GCN (aggregator 'gcn') + linear head on 8 Trainium2 cores.

Approach (hardcoded for this problem's sizes):
  - Both layers are dst-sharded: layer 1's 131072 dst nodes -> 16384/core,
    layer 2's 8192 -> 1024/core. Each core's dsts are permuted into 128-wide
    blocks with degree-balanced (serpentine) binning so every block has a
    near-equal edge count; a single compile-time KMAX bounds chunks/block.
  - Per block, the host lays out a private gather-table region holding the
    distinct source rows that block references; the device fetches every
    edge's source row with dma_gather (int16 region-local indices, four SWDGE
    queues in parallel).
  - Segment-sum runs on the tensor engine: per 128-edge chunk, a one-hot
    (edge x dst) matrix built on the vector engine is the stationary operand;
    gathered rows stream through, accumulating neigh sums in PSUM. A paired
    rhs=ones matmul accumulates degrees.
  - normalize by 1/(deg+1), add self features, fc (+bias, relu) on
    PE/DVE/ACT, layer-2 adds the 64-wide linear head. Outputs return
    transposed; the host reassembles and un-permutes.

HW time is dominated by the layer-1 edge gather (2M x 512 B rows).
"""

import numpy as np

import concourse.bass as bass
import concourse.bacc as bacc
import concourse.mybir as mybir
import concourse.tile as tile
from concourse import bass_utils
from concourse.masks import make_identity

F32 = mybir.dt.float32
I16 = mybir.dt.int16

N0, IN = 1048576, 128
E0, ND0 = 2097152, 131072
E1, ND1 = 131072, 8192
HID, OUTF, PHEAD = 256, 256, 64
NCORES = 8
P = 128
NB1 = 128   # dst blocks per core, layer 1
NB2 = 8     # dst blocks per core, layer 2

TRACE = False  # test harness may flip this for profiling


# ----------------------------------------------------------------------------
# Host-side scheduling
# ----------------------------------------------------------------------------

def _schedule(dst_arr, nd, nb):
    """Assign each dst id to (core, block, slot); every block holds exactly P
    dsts with near-equal total degree (serpentine deal by degree)."""
    deg = np.bincount(dst_arr, minlength=nd)
    dorder = np.argsort(-deg, kind="stable")
    i = np.arange(nd)
    r, pos = divmod(i, NCORES)
    serp = np.where(r % 2 == 0, pos, NCORES - 1 - pos)
    core_of = np.empty(nd, np.int64)
    core_of[dorder] = serp

    block_of = np.empty(nd, np.int64)
    dstids_blocked = np.empty((NCORES, nb * P), np.int64)
    for c in range(NCORES):
        ids = dorder[core_of[dorder] == c]
        m = ids.size
        assert m == nb * P, (m, nb, P)
        j = np.arange(m)
        rb, pb = divmod(j, nb)
        serpb = np.where(rb % 2 == 0, pb, nb - 1 - pb)
        block_of[ids] = serpb
        ord2 = np.lexsort((rb, serpb))
        dstids_blocked[c] = ids[ord2]
    return core_of, block_of, dstids_blocked


def _edge_layout(src_arr, dst_arr, x_table, nd, nb):
    """Per-core device arrays for one layer.

    Returns dict with per-core lists:
      xtab [nb*RCAP, fin]  region tables (block b's distinct src rows at
                           rows [b*RCAP, ...))
      idxw [P, nb*kmax*8]  int16 wrapped region-local gather indices
      dstl [P, nb*kmax]    f32 dst slot within block (-1 dummy)
      xself [nb*P, fin]    self features in block order
    plus dstids_blocked and kmax.
    Edge slot i of block b maps to gather out (p=i%128, col=i//128); the
    matmul chunk j consumes slots {j*128+p}.
    """
    fin = x_table.shape[1]
    core_of, block_of, dstids_blocked = _schedule(dst_arr, nd, nb)
    slot_of = np.empty(nd, np.int64)
    for c in range(NCORES):
        slot_of[dstids_blocked[c]] = np.arange(nb * P) % P

    eco = core_of[dst_arr]
    ebl = block_of[dst_arr]
    esl = slot_of[dst_arr]

    # kmax: max edges in any (core, block)
    bsizes = np.bincount(eco * nb + ebl, minlength=NCORES * nb)
    kmax = int(np.ceil(bsizes.max() / P))
    rcap = kmax * P

    out = {"xtab": [], "idxw": [], "dstl": [], "xself": []}
    for c in range(NCORES):
        mask = eco == c
        s_e = src_arr[mask]
        b_e = ebl[mask]
        d_e = esl[mask]
        ne = s_e.size

        # slot assignment: rank within block (original order)
        border = np.argsort(b_e, kind="stable")
        bs = np.bincount(b_e, minlength=nb)
        starts = np.zeros(nb + 1, np.int64)
        np.cumsum(bs, out=starts[1:])
        rank = np.arange(ne) - starts[b_e[border]]
        # slot i = rank; gather layout (p = i % P, j = i // P)
        jj, pp = divmod(rank, P)

        # distinct (block, src) pairs -> region-local ids
        o2 = np.lexsort((s_e, b_e))
        k_b, k_s = b_e[o2], s_e[o2]
        first = np.ones(ne, bool)
        first[1:] = (k_b[1:] != k_b[:-1]) | (k_s[1:] != k_s[:-1])
        gid_sorted = np.cumsum(first) - 1
        dblk = k_b[first]
        dsrc = k_s[first]
        nd_ct = np.bincount(dblk, minlength=nb)
        dbase = np.zeros(nb + 1, np.int64)
        np.cumsum(nd_ct, out=dbase[1:])
        local_d = np.arange(dsrc.size) - dbase[dblk]
        assert local_d.max() < rcap
        # per-edge local idx (in o2 order), then scatter to slots
        edge_local_o2 = local_d[gid_sorted]
        edge_local = np.empty(ne, np.int64)
        edge_local[o2] = edge_local_o2

        seq = np.zeros((nb, rcap), np.int16)
        seq[b_e[border], jj * P + pp] = edge_local[border]
        dstl = np.full((nb, rcap), -1.0, np.float32)
        dstl[b_e[border], jj * P + pp] = d_e[border]

        # wrapped idx: per block [128, rcap//16]
        w = seq.reshape(nb, rcap // 16, 16).transpose(0, 2, 1)  # [nb, 16, rcap/16]
        idxw = np.tile(w, (1, 8, 1)).transpose(1, 0, 2).reshape(P, nb * (rcap // 16))

        # dstl in device layout [P, nb*kmax]: dstl_dev[p, b*kmax+j] = dstl[b, j*128+p]
        dstl_dev = dstl.reshape(nb, kmax, P).transpose(2, 0, 1).reshape(P, nb * kmax)

        xtab = np.zeros((nb * rcap, fin), np.float32)
        xtab[dblk * rcap + local_d] = x_table[dsrc]

        out["xtab"].append(xtab)
        out["idxw"].append(np.ascontiguousarray(idxw))
        out["dstl"].append(np.ascontiguousarray(dstl_dev))
        out["xself"].append(np.ascontiguousarray(x_table[dstids_blocked[c]]))
    return out, dstids_blocked, kmax


# ----------------------------------------------------------------------------
# Device program
# ----------------------------------------------------------------------------

def _build_sage_layer(nb, kmax, fin, fout, head=False):
    nc = bacc.Bacc("TRN2", target_bir_lowering=False, debug=False,
                   num_devices=NCORES, num_swdge_queues=4)
    rcap = kmax * P
    xtab = nc.dram_tensor("xtab", [nb * rcap, fin], F32, kind="ExternalInput").ap()
    idx_t = nc.dram_tensor("idx", [P, nb * (rcap // 16)], I16, kind="ExternalInput").ap()
    dstl_t = nc.dram_tensor("dstl", [P, nb * kmax], F32, kind="ExternalInput").ap()
    xself_t = nc.dram_tensor("xself", [nb * P, fin], F32, kind="ExternalInput").ap()
    wT_t = nc.dram_tensor("wT", [fin, fout], F32, kind="ExternalInput").ap()
    br_t = nc.dram_tensor("br", [P, fout // P], F32, kind="ExternalInput").ap()
    if head:
        whT_t = nc.dram_tensor("whT", [fout, PHEAD], F32, kind="ExternalInput").ap()
        bhr_t = nc.dram_tensor("bhr", [PHEAD, 1], F32, kind="ExternalInput").ap()
        out_t = nc.dram_tensor("outT", [PHEAD, nb * P], F32, kind="ExternalOutput").ap()
    else:
        out_t = nc.dram_tensor("hT", [fout, nb * P], F32, kind="ExternalOutput").ap()

    foc = fout // P
    fic = fin // P

    with tile.TileContext(nc) as tc:
        with tc.tile_pool(name="const", bufs=1) as cpool, \
             tc.tile_pool(name="idxp", bufs=1) as ixpool, \
             tc.tile_pool(name="g", bufs=6) as gpool, \
             tc.tile_pool(name="oh", bufs=4) as ohpool, \
             tc.tile_pool(name="sb", bufs=3) as spool, \
             tc.tile_pool(name="pseg", bufs=2, space="PSUM") as pspool, \
             tc.tile_pool(name="pdeg", bufs=2, space="PSUM") as pdpool, \
             tc.tile_pool(name="ptp", bufs=2, space="PSUM") as tppool, \
             tc.tile_pool(name="pfc", bufs=2, space="PSUM") as fcpool:

            ident = cpool.tile([P, P], F32)
            make_identity(nc, ident[:])
            ones_t = cpool.tile([P, 1], F32, tag="ones")
            nc.vector.memset(ones_t[:], 1.0)
            wt_tiles = []
            for kc in range(fic):
                t = cpool.tile([P, fout], F32, tag=f"wt{kc}")
                nc.sync.dma_start(t[:], wT_t[kc * P:(kc + 1) * P, :])
                wt_tiles.append(t)
            bt = cpool.tile([P, foc], F32, tag="bt")
            nc.sync.dma_start(bt[:], br_t)
            if head:
                wh_tiles = []
                for kc in range(foc):
                    t = cpool.tile([P, PHEAD], F32, tag=f"wh{kc}")
                    nc.sync.dma_start(t[:], whT_t[kc * P:(kc + 1) * P, :])
                    wh_tiles.append(t)
                bh_tile = cpool.tile([PHEAD, 1], F32, tag="bh")
                nc.sync.dma_start(bh_tile[:], bhr_t)

            iotai = cpool.tile([P, P], mybir.dt.int32, tag="ioti")
            nc.gpsimd.iota(iotai[:], pattern=[[1, P]], channel_multiplier=0)
            iotaf = cpool.tile([P, P], F32, tag="iotf")
            nc.vector.tensor_copy(iotaf[:], iotai[:])
            iotaf_b = iotaf[:].rearrange("p (o d) -> p o d", o=1).to_broadcast(
                [P, kmax, P])

            idxt = ixpool.tile([P, nb * (rcap // 16)], I16, tag="idx")
            nc.sync.dma_start(idxt[:], idx_t)
            dslt = ixpool.tile([P, nb * kmax], F32, tag="dsl")
            nc.sync.dma_start(dslt[:], dstl_t)

            for b in range(nb):
                g = gpool.tile([P, kmax, fin], F32, tag="g")
                nc.gpsimd.dma_gather(
                    out_ap=g[:],
                    in_ap=xtab[b * rcap:(b + 1) * rcap, :],
                    idxs_ap=idxt[:, b * (rcap // 16):(b + 1) * (rcap // 16)],
                    num_idxs=rcap,
                    num_idxs_reg=rcap,
                    elem_size=fin,
                    single_packet=False,
                    queue_num=b % 4,
                )

                oh = ohpool.tile([P, kmax, P], F32, tag="oh")
                dsl_b = dslt[:, b * kmax:(b + 1) * kmax].rearrange(
                    "p (k o) -> p k o", o=1).to_broadcast([P, kmax, P])
                nc.vector.tensor_tensor(out=oh[:], in0=dsl_b, in1=iotaf_b,
                                        op=mybir.AluOpType.is_equal)

                ps = pspool.tile([P, fin], F32, tag="pseg")
                psd = pdpool.tile([P, 1], F32, tag="pdeg")
                for j in range(kmax):
                    nc.tensor.matmul(out=ps[:], lhsT=oh[:, j, :], rhs=g[:, j, :],
                                     start=(j == 0), stop=(j == kmax - 1))
                    nc.tensor.matmul(out=psd[:], lhsT=oh[:, j, :],
                                     rhs=ones_t[:, 0:1],
                                     start=(j == 0), stop=(j == kmax - 1))

                xs = spool.tile([P, fin], F32, tag="xs")
                nc.sync.dma_start(xs[:], xself_t[b * P:(b + 1) * P, :])
                s = spool.tile([P, fin], F32, tag="s")
                nc.vector.tensor_add(s[:], ps[:], xs[:])
                d1 = spool.tile([P, 1], F32, tag="d1")
                nc.vector.tensor_scalar_add(d1[:], psd[:], 1.0)
                rc = spool.tile([P, 1], F32, tag="rc")
                nc.vector.reciprocal(rc[:], d1[:])
                hn = spool.tile([P, fin], F32, tag="hn")
                nc.vector.tensor_scalar_mul(hn[:], s[:], rc[:, 0:1])

                hnT_tiles = []
                for kc in range(fic):
                    tp = tppool.tile([P, P], F32, tag="tp")
                    nc.tensor.transpose(tp[:], hn[:, kc * P:(kc + 1) * P], ident[:])
                    ht = spool.tile([P, P], F32, tag=f"hnT{kc}")
                    nc.vector.tensor_copy(ht[:], tp[:])
                    hnT_tiles.append(ht)

                h_tiles = []
                for oc in range(foc):
                    pf = fcpool.tile([P, P], F32, tag="pfc")
                    for kc in range(fic):
                        nc.tensor.matmul(
                            out=pf[:],
                            lhsT=wt_tiles[kc][:, oc * P:(oc + 1) * P],
                            rhs=hnT_tiles[kc][:],
                            start=(kc == 0), stop=(kc == fic - 1))
                    hs = spool.tile([P, P], F32, tag=f"hs{oc}")
                    nc.scalar.activation(hs[:], pf[:],
                                         mybir.ActivationFunctionType.Relu,
                                         bias=bt[:, oc:oc + 1], scale=1.0)
                    h_tiles.append(hs)
                    if not head:
                        nc.sync.dma_start(
                            out_t[oc * P:(oc + 1) * P, b * P:(b + 1) * P], hs[:])

                if head:
                    ph = fcpool.tile([PHEAD, P], F32, tag="pfc")
                    for kc in range(foc):
                        nc.tensor.matmul(out=ph[:], lhsT=wh_tiles[kc][:],
                                         rhs=h_tiles[kc][:],
                                         start=(kc == 0), stop=(kc == foc - 1))
                    os_ = spool.tile([PHEAD, P], F32, tag="os")
                    nc.vector.tensor_scalar_add(os_[:], ph[:], bh_tile[:, 0:1])
                    nc.sync.dma_start(out_t[:, b * P:(b + 1) * P], os_[:])

    nc.compile()
    return nc


# ----------------------------------------------------------------------------
# Host orchestration
# ----------------------------------------------------------------------------

def _run_layer(feat_table, src_arr, dst_arr, nd, nb, w, bvec, head_w=None,
               head_b=None, debug=None, tag=""):
    fin = feat_table.shape[1]
    fout = w.shape[0]
    arrs, dstids_blocked, kmax = _edge_layout(src_arr, dst_arr, feat_table, nd, nb)

    wT = np.ascontiguousarray(w.T).astype(np.float32)
    br = np.ascontiguousarray(bvec.reshape(fout // P, P).T)

    in_maps = []
    for c in range(NCORES):
        m = {
            "xtab": arrs["xtab"][c],
            "idx": arrs["idxw"][c],
            "dstl": arrs["dstl"][c],
            "xself": arrs["xself"][c],
            "wT": wT,
            "br": br,
        }
        if head_w is not None:
            m["whT"] = np.ascontiguousarray(head_w.T).astype(np.float32)
            m["bhr"] = np.ascontiguousarray(head_b.reshape(PHEAD, 1)).astype(np.float32)
        in_maps.append(m)

    nc = _build_sage_layer(nb, kmax, fin, fout, head=head_w is not None)
    res = bass_utils.run_bass_kernel_spmd(
        nc, in_maps, core_ids=list(range(NCORES)), trace=TRACE)
    if debug is not None:
        debug.setdefault("exec_ns", {})[tag] = res.exec_time_ns
        debug.setdefault("trace", {})[tag] = (
            None if res.instructions_and_trace is None
            else res.instructions_and_trace[1])

    outname = "outT" if head_w is not None else "hT"
    outdim = PHEAD if head_w is not None else fout
    full = np.empty((nd, outdim), np.float32)
    for c in range(NCORES):
        full[dstids_blocked[c]] = res.results[c][outname].T
    return full


def kernel(x, src0, dst0, src1, dst1, W1, b1, W2, b2, Wh, bh,
           n_dst0, n_dst1, task_index, _debug=None):
    x = np.asarray(x, np.float32)
    src0 = np.asarray(src0).astype(np.int64)
    dst0 = np.asarray(dst0).astype(np.int64)
    src1 = np.asarray(src1).astype(np.int64)
    dst1 = np.asarray(dst1).astype(np.int64)
    W1 = np.asarray(W1, np.float32); b1 = np.asarray(b1, np.float32)
    W2 = np.asarray(W2, np.float32); b2 = np.asarray(b2, np.float32)
    Wh = np.asarray(Wh, np.float32); bh = np.asarray(bh, np.float32)

    h1 = _run_layer(x, src0, dst0, ND0, NB1, W1, b1, debug=_debug, tag="l1")
    out = _run_layer(h1, src1, dst1, ND1, NB2, W2, b2,
                     head_w=Wh, head_b=bh, debug=_debug, tag="l2")
    return out


# revision 8
# speedup vs baseline: 2.9941x; 2.9941x over previous
"""Two-layer GraphSAGE-GCN (aggregator 'gcn') + linear head on 8 Trainium2 cores.

Approach (hardcoded for this problem's sizes):
  - Both layers are dst-sharded: layer 1's 131072 dst nodes -> 16384/core,
    layer 2's 8192 -> 1024/core. Each core's dsts are permuted into 128-wide
    blocks with degree-balanced (serpentine) binning so every block has a
    near-equal edge count; a single compile-time KMAX bounds chunks/block.
  - Per block, the host lays out a private gather-table region holding the
    distinct source rows that block references; the device fetches every
    edge's source row with dma_gather (int16 region-local indices, four SWDGE
    queues in parallel).
  - Segment-sum runs on the tensor engine: per 128-edge chunk, a one-hot
    (edge x dst) matrix built on the vector engine is the stationary operand;
    gathered rows stream through, accumulating neigh sums in PSUM. A paired
    rhs=ones matmul accumulates degrees.
  - normalize by 1/(deg+1), add self features, fc (+bias, relu) on
    PE/DVE/ACT, layer-2 adds the 64-wide linear head. Outputs return
    transposed; the host reassembles and un-permutes.

HW time is dominated by the layer-1 edge gather (2M x 512 B rows).
"""

import numpy as np

import concourse.bass as bass
import concourse.bacc as bacc
import concourse.mybir as mybir
import concourse.tile as tile
from concourse import bass_utils
from concourse.masks import make_identity


def _dma_gather_raw(gp, out_ap, in_ap, idxs_ap, num_idxs, elem_size, elem_step,
                    queue_num):
    """bass.BassGpSimd.dma_gather for the non-transpose HBM path, minus the
    %256 elem-size assert (that restriction only applies to transpose mode;
    the Q7 generator handles arbitrary descriptor lengths)."""
    import concourse.ap_utils as ap_utils
    assert idxs_ap.dtype == mybir.dt.int16
    assert in_ap.dtype == out_ap.dtype
    assert ap_utils.ap_is_contiguous(out_ap.ap[1:])
    assert ap_utils.ap_is_contiguous(idxs_ap.ap[1:])
    assert in_ap.ap[0][0] == elem_step
    stride_bytes = elem_step * mybir.dt.size(in_ap.dtype)
    stride_bytes_256 = stride_bytes // 256
    assert stride_bytes % 256 == 0 and stride_bytes_256 < 256
    _in_ap = gp.lower_ap_dma(in_ap, for_custom_bir_dma=True)
    _idxs_ap = gp.lower_ap(idxs_ap)
    _out_ap = gp.lower_ap(out_ap)
    return gp.add_instruction(
        mybir.InstDMAGatherAnt(
            name=gp.bass.get_next_instruction_name(),
            ins=[*_in_ap, _idxs_ap, gp.lower_val_access(gp.to_reg(num_idxs))],
            outs=[_out_ap],
            transpose=False,
            num_idxs=num_idxs,
            elem_size=elem_size,
            stride_bytes_256=stride_bytes_256,
            gen_mode=0,
            single_packet=False,
            queue_num=queue_num,
            sbuf_tokens_per_rank=0,
            sbuf_free_dim_per_rank=0,
            sbuf_free_dim_pad_per_rank=0,
            sbuf_byte_offset=0,
        ))

F32 = mybir.dt.float32
F32R = mybir.dt.float32r
I16 = mybir.dt.int16

N0, IN = 1048576, 128
E0, ND0 = 2097152, 131072
E1, ND1 = 131072, 8192
HID, OUTF, PHEAD = 256, 256, 64
NCORES = 8
P = 128
NB1 = 128   # dst blocks per core, layer 1
NB2 = 8     # dst blocks per core, layer 2

TRACE = False  # test harness may flip this for profiling


# ----------------------------------------------------------------------------
# Host-side scheduling
# ----------------------------------------------------------------------------

def _schedule(dst_arr, nd, nb):
    """Assign each dst id to (core, block, slot); every block holds exactly P
    dsts with near-equal total degree (serpentine deal by degree)."""
    deg = np.bincount(dst_arr, minlength=nd)
    dorder = np.argsort(-deg, kind="stable")
    i = np.arange(nd)
    r, pos = divmod(i, NCORES)
    serp = np.where(r % 2 == 0, pos, NCORES - 1 - pos)
    core_of = np.empty(nd, np.int64)
    core_of[dorder] = serp

    block_of = np.empty(nd, np.int64)
    dstids_blocked = np.empty((NCORES, nb * P), np.int64)
    for c in range(NCORES):
        ids = dorder[core_of[dorder] == c]
        m = ids.size
        assert m == nb * P, (m, nb, P)
        j = np.arange(m)
        rb, pb = divmod(j, nb)
        serpb = np.where(rb % 2 == 0, pb, nb - 1 - pb)
        block_of[ids] = serpb
        ord2 = np.lexsort((rb, serpb))
        dstids_blocked[c] = ids[ord2]
    return core_of, block_of, dstids_blocked


def _edge_layout(src_arr, dst_arr, x_table, nd, nb):
    """Per-core device arrays for one layer.

    Returns dict with per-core lists:
      xtab [nb*RCAP, fin]  region tables (block b's distinct src rows at
                           rows [b*RCAP, ...))
      idxw [P, nb*kmax*8]  int16 wrapped region-local gather indices
      dstl [P, nb*kmax]    f32 dst slot within block (-1 dummy)
      xself [nb*P, fin]    self features in block order
    plus dstids_blocked and kmax.
    Edge slot i of block b maps to gather out (p=i%128, col=i//128); the
    matmul chunk j consumes slots {j*128+p}.
    """
    fin = x_table.shape[1]
    core_of, block_of, dstids_blocked = _schedule(dst_arr, nd, nb)
    slot_of = np.empty(nd, np.int64)
    for c in range(NCORES):
        slot_of[dstids_blocked[c]] = np.arange(nb * P) % P

    eco = core_of[dst_arr]
    ebl = block_of[dst_arr]
    esl = slot_of[dst_arr]

    # kmax: max edges in any (core, block)
    bsizes = np.bincount(eco * nb + ebl, minlength=NCORES * nb)
    kmax = int(np.ceil(bsizes.max() / P))
    rcap = kmax * P

    out = {"xtab": [], "idxw": [], "dstl": [], "xself": []}
    for c in range(NCORES):
        mask = eco == c
        s_e = src_arr[mask]
        b_e = ebl[mask]
        d_e = esl[mask]
        ne = s_e.size

        # slot assignment: rank within block (original order)
        border = np.argsort(b_e, kind="stable")
        bs = np.bincount(b_e, minlength=nb)
        starts = np.zeros(nb + 1, np.int64)
        np.cumsum(bs, out=starts[1:])
        rank = np.arange(ne) - starts[b_e[border]]
        # slot i = rank; gather layout (p = i % P, j = i // P)
        jj, pp = divmod(rank, P)

        # distinct (block, src) pairs -> region-local ids
        o2 = np.lexsort((s_e, b_e))
        k_b, k_s = b_e[o2], s_e[o2]
        first = np.ones(ne, bool)
        first[1:] = (k_b[1:] != k_b[:-1]) | (k_s[1:] != k_s[:-1])
        gid_sorted = np.cumsum(first) - 1
        dblk = k_b[first]
        dsrc = k_s[first]
        nd_ct = np.bincount(dblk, minlength=nb)
        dbase = np.zeros(nb + 1, np.int64)
        np.cumsum(nd_ct, out=dbase[1:])
        local_d = np.arange(dsrc.size) - dbase[dblk]
        assert local_d.max() < rcap
        # per-edge local idx (in o2 order), then scatter to slots
        edge_local_o2 = local_d[gid_sorted]
        edge_local = np.empty(ne, np.int64)
        edge_local[o2] = edge_local_o2

        seq = np.zeros((nb, rcap), np.int16)
        seq[b_e[border], jj * P + pp] = edge_local[border]
        dstl = np.full((nb, rcap), -1.0, np.float32)
        dstl[b_e[border], jj * P + pp] = d_e[border]

        # wrapped idx: per block [128, rcap//16]
        w = seq.reshape(nb, rcap // 16, 16).transpose(0, 2, 1)  # [nb, 16, rcap/16]
        idxw = np.tile(w, (1, 8, 1)).transpose(1, 0, 2).reshape(P, nb * (rcap // 16))

        # dstl in device layout [P, nb*kmax]: dstl_dev[p, b*kmax+j] = dstl[b, j*128+p]
        dstl_dev = dstl.reshape(nb, kmax, P).transpose(2, 0, 1).reshape(P, nb * kmax)

        rstride = (fin + 64 // 1) // 64 * 64 + 64  # fin+1 padded to 64-elem (256B) multiple
        rstride = ((fin + 1 + 63) // 64) * 64
        xtab = np.zeros((nb * rcap, rstride), np.float32)
        xtab[dblk * rcap + local_d, :fin] = x_table[dsrc]
        xtab[:, fin] = 1.0

        out["xtab"].append(xtab)
        out["idxw"].append(np.ascontiguousarray(idxw))
        out["dstl"].append(np.ascontiguousarray(dstl_dev))
        out["xself"].append(np.ascontiguousarray(x_table[dstids_blocked[c]]))
    return out, dstids_blocked, kmax


# ----------------------------------------------------------------------------
# Device program
# ----------------------------------------------------------------------------

def _build_sage_layer(nb, kmax, fin, fout, head=False):
    nc = bacc.Bacc("TRN2", target_bir_lowering=False, debug=False,
                   num_devices=NCORES, num_swdge_queues=4)
    rcap = kmax * P
    rstride = ((fin + 1 + 63) // 64) * 64
    xtab = nc.dram_tensor("xtab", [nb * rcap, rstride], F32R, kind="ExternalInput").ap()
    idx_t = nc.dram_tensor("idx", [P, nb * (rcap // 16)], I16, kind="ExternalInput").ap()
    dstl_t = nc.dram_tensor("dstl", [P, nb * kmax], F32, kind="ExternalInput").ap()
    xself_t = nc.dram_tensor("xself", [nb * P, fin], F32, kind="ExternalInput").ap()
    wT_t = nc.dram_tensor("wT", [fin, fout], F32R, kind="ExternalInput").ap()
    br_t = nc.dram_tensor("br", [P, fout // P], F32, kind="ExternalInput").ap()
    if head:
        whT_t = nc.dram_tensor("whT", [fout, PHEAD], F32R, kind="ExternalInput").ap()
        bhr_t = nc.dram_tensor("bhr", [PHEAD, 1], F32, kind="ExternalInput").ap()
        out_t = nc.dram_tensor("outT", [PHEAD, nb * P], F32, kind="ExternalOutput").ap()
    else:
        out_t = nc.dram_tensor("hT", [fout, nb * P], F32, kind="ExternalOutput").ap()

    foc = fout // P
    fic = fin // P

    with tile.TileContext(nc) as tc:
        with tc.tile_pool(name="const", bufs=1) as cpool, \
             tc.tile_pool(name="idxp", bufs=1) as ixpool, \
             tc.tile_pool(name="g", bufs=6) as gpool, \
             tc.tile_pool(name="oh", bufs=4) as ohpool, \
             tc.tile_pool(name="sb", bufs=3) as spool, \
             tc.tile_pool(name="pseg", bufs=2, space="PSUM") as pspool, \
             tc.tile_pool(name="ptp", bufs=2, space="PSUM") as tppool, \
             tc.tile_pool(name="pfc", bufs=2, space="PSUM") as fcpool:

            ident = cpool.tile([P, P], F32)
            make_identity(nc, ident[:])
            ones_t = cpool.tile([P, 1], F32, tag="ones")
            nc.vector.memset(ones_t[:], 1.0)
            wt_tiles = []
            for kc in range(fic):
                t = cpool.tile([P, fout], F32R, tag=f"wt{kc}")
                nc.sync.dma_start(t[:], wT_t[kc * P:(kc + 1) * P, :])
                wt_tiles.append(t)
            bt = cpool.tile([P, foc], F32, tag="bt")
            nc.sync.dma_start(bt[:], br_t)
            if head:
                wh_tiles = []
                for kc in range(foc):
                    t = cpool.tile([P, PHEAD], F32R, tag=f"wh{kc}")
                    nc.sync.dma_start(t[:], whT_t[kc * P:(kc + 1) * P, :])
                    wh_tiles.append(t)
                bh_tile = cpool.tile([PHEAD, 1], F32, tag="bh")
                nc.sync.dma_start(bh_tile[:], bhr_t)

            iotai = cpool.tile([P, P], mybir.dt.int32, tag="ioti")
            nc.gpsimd.iota(iotai[:], pattern=[[1, P]], channel_multiplier=0)
            iotaf = cpool.tile([P, P], F32, tag="iotf")
            nc.vector.tensor_copy(iotaf[:], iotai[:])
            iotaf_b = iotaf[:].rearrange("p (o d) -> p o d", o=1).to_broadcast(
                [P, kmax, P])

            idxt = ixpool.tile([P, nb * (rcap // 16)], I16, tag="idx")
            nc.sync.dma_start(idxt[:], idx_t)
            dslt = ixpool.tile([P, nb * kmax], F32, tag="dsl")
            nc.sync.dma_start(dslt[:], dstl_t)

            for b in range(nb):
                g = gpool.tile([P, kmax, fin + 2], F32R, tag="g")
                _dma_gather_raw(
                    nc.gpsimd,
                    out_ap=g[:],
                    in_ap=xtab[b * rcap:(b + 1) * rcap, :],
                    idxs_ap=idxt[:, b * (rcap // 16):(b + 1) * (rcap // 16)],
                    num_idxs=rcap,
                    elem_size=fin + 2,
                    elem_step=rstride,
                    queue_num=b % 4,
                )

                oh = ohpool.tile([P, kmax, P], F32R, tag="oh")
                dsl_b = dslt[:, b * kmax:(b + 1) * kmax].rearrange(
                    "p (k o) -> p k o", o=1).to_broadcast([P, kmax, P])
                nc.vector.tensor_tensor(out=oh[:], in0=dsl_b, in1=iotaf_b,
                                        op=mybir.AluOpType.is_equal)

                ps = pspool.tile([P, fin + 2], F32, tag="pseg")
                for j in range(kmax):
                    nc.tensor.matmul(out=ps[:], lhsT=oh[:, j, :],
                                     rhs=g[:, j, :],
                                     start=(j == 0), stop=(j == kmax - 1))

                xs = spool.tile([P, fin], F32, tag="xs")
                nc.sync.dma_start(xs[:], xself_t[b * P:(b + 1) * P, :])
                s = spool.tile([P, fin], F32, tag="s")
                nc.vector.tensor_add(s[:], ps[:, 0:fin], xs[:])
                d1 = spool.tile([P, 1], F32, tag="d1")
                nc.vector.tensor_add(d1[:], ps[:, fin:fin + 1], ones_t[:])
                rc = spool.tile([P, 1], F32, tag="rc")
                nc.vector.reciprocal(rc[:], d1[:])
                hn = spool.tile([P, fin], F32, tag="hn")
                rc_b = rc[:].to_broadcast([P, fin])
                nc.vector.tensor_tensor(out=hn[:], in0=s[:], in1=rc_b,
                                        op=mybir.AluOpType.mult)

                hnT_tiles = []
                for kc in range(fic):
                    tp = tppool.tile([P, P], F32, tag="tp")
                    nc.tensor.transpose(tp[:], hn[:, kc * P:(kc + 1) * P], ident[:])
                    ht = spool.tile([P, P], F32R, tag=f"hnT{kc}")
                    nc.vector.tensor_copy(ht[:], tp[:])
                    hnT_tiles.append(ht)

                h_tiles = []
                for oc in range(foc):
                    pf = fcpool.tile([P, P], F32, tag="pfc")
                    for kc in range(fic):
                        nc.tensor.matmul(
                            out=pf[:],
                            lhsT=wt_tiles[kc][:, oc * P:(oc + 1) * P],
                            rhs=hnT_tiles[kc][:],
                            start=(kc == 0), stop=(kc == fic - 1))
                    hs = spool.tile([P, P], F32R, tag=f"hs{oc}")
                    nc.scalar.activation(hs[:], pf[:],
                                         mybir.ActivationFunctionType.Relu,
                                         bias=bt[:, oc:oc + 1], scale=1.0)
                    h_tiles.append(hs)
                    if not head:
                        nc.sync.dma_start(
                            out_t[oc * P:(oc + 1) * P, b * P:(b + 1) * P],
                            hs[:].bitcast(F32))

                if head:
                    ph = fcpool.tile([PHEAD, P], F32, tag="pfc")
                    for kc in range(foc):
                        nc.tensor.matmul(out=ph[:],
                                         lhsT=wh_tiles[kc][:],
                                         rhs=h_tiles[kc][:],
                                         start=(kc == 0), stop=(kc == foc - 1))
                    os_ = spool.tile([PHEAD, P], F32, tag="os")
                    nc.vector.tensor_scalar_add(os_[:], ph[:], bh_tile[:, 0:1])
                    nc.sync.dma_start(out_t[:, b * P:(b + 1) * P], os_[:])

    nc.compile()
    return nc


# ----------------------------------------------------------------------------
# Host orchestration
# ----------------------------------------------------------------------------

def _run_layer(feat_table, src_arr, dst_arr, nd, nb, w, bvec, head_w=None,
               head_b=None, debug=None, tag=""):
    fin = feat_table.shape[1]
    fout = w.shape[0]
    arrs, dstids_blocked, kmax = _edge_layout(src_arr, dst_arr, feat_table, nd, nb)

    wT = np.ascontiguousarray(w.T).astype(np.float32)
    br = np.ascontiguousarray(bvec.reshape(fout // P, P).T)

    in_maps = []
    for c in range(NCORES):
        m = {
            "xtab": arrs["xtab"][c],
            "idx": arrs["idxw"][c],
            "dstl": arrs["dstl"][c],
            "xself": arrs["xself"][c],
            "wT": wT,
            "br": br,
        }
        if head_w is not None:
            m["whT"] = np.ascontiguousarray(head_w.T).astype(np.float32)
            m["bhr"] = np.ascontiguousarray(head_b.reshape(PHEAD, 1)).astype(np.float32)
        in_maps.append(m)

    nc = _build_sage_layer(nb, kmax, fin, fout, head=head_w is not None)
    res = bass_utils.run_bass_kernel_spmd(
        nc, in_maps, core_ids=list(range(NCORES)), trace=TRACE)
    if debug is not None:
        debug.setdefault("exec_ns", {})[tag] = res.exec_time_ns
        debug.setdefault("trace", {})[tag] = (
            None if res.instructions_and_trace is None
            else res.instructions_and_trace[1])

    outname = "outT" if head_w is not None else "hT"
    outdim = PHEAD if head_w is not None else fout
    full = np.empty((nd, outdim), np.float32)
    for c in range(NCORES):
        full[dstids_blocked[c]] = res.results[c][outname].T
    return full


def kernel(x, src0, dst0, src1, dst1, W1, b1, W2, b2, Wh, bh,
           n_dst0, n_dst1, task_index, _debug=None):
    x = np.asarray(x, np.float32)
    src0 = np.asarray(src0).astype(np.int64)
    dst0 = np.asarray(dst0).astype(np.int64)
    src1 = np.asarray(src1).astype(np.int64)
    dst1 = np.asarray(dst1).astype(np.int64)
    W1 = np.asarray(W1, np.float32); b1 = np.asarray(b1, np.float32)
    W2 = np.asarray(W2, np.float32); b2 = np.asarray(b2, np.float32)
    Wh = np.asarray(Wh, np.float32); bh = np.asarray(bh, np.float32)

    h1 = _run_layer(x, src0, dst0, ND0, NB1, W1, b1, debug=_debug, tag="l1")
    out = _run_layer(h1, src1, dst1, ND1, NB2, W2, b2,
                     head_w=Wh, head_b=bh, debug=_debug, tag="l2")
    return out


# revision 10
# speedup vs baseline: 3.2928x; 1.0998x over previous
"""Two-layer GraphSAGE-GCN (aggregator 'gcn') + linear head on 8 Trainium2 cores.

Approach (hardcoded for this problem's sizes):
  - Both layers are dst-sharded: layer 1's 131072 dst nodes -> 16384/core,
    layer 2's 8192 -> 1024/core. Each core's dsts are permuted into 128-wide
    blocks with degree-balanced (serpentine) binning so every block has a
    near-equal edge count; a single compile-time KMAX bounds chunks/block.
  - Per block, the host lays out a private gather-table region holding the
    distinct source rows that block references; the device fetches every
    edge's source row with dma_gather (int16 region-local indices, four SWDGE
    queues in parallel).
  - Segment-sum runs on the tensor engine: per 128-edge chunk, a one-hot
    (edge x dst) matrix built on the vector engine is the stationary operand;
    gathered rows stream through, accumulating neigh sums in PSUM. A paired
    rhs=ones matmul accumulates degrees.
  - normalize by 1/(deg+1), add self features, fc (+bias, relu) on
    PE/DVE/ACT, layer-2 adds the 64-wide linear head. Outputs return
    transposed; the host reassembles and un-permutes.

HW time is dominated by the layer-1 edge gather (2M x 512 B rows).
"""

import numpy as np

import concourse.bass as bass
import concourse.bacc as bacc
import concourse.mybir as mybir
import concourse.tile as tile
from concourse import bass_utils
from concourse.masks import make_identity


def _dma_gather_raw(gp, out_ap, in_ap, idxs_ap, num_idxs, elem_size, elem_step,
                    queue_num):
    """bass.BassGpSimd.dma_gather for the non-transpose HBM path, minus the
    %256 elem-size assert (that restriction only applies to transpose mode;
    the Q7 generator handles arbitrary descriptor lengths)."""
    import concourse.ap_utils as ap_utils
    assert idxs_ap.dtype == mybir.dt.int16
    assert in_ap.dtype == out_ap.dtype
    assert ap_utils.ap_is_contiguous(out_ap.ap[1:])
    assert ap_utils.ap_is_contiguous(idxs_ap.ap[1:])
    assert in_ap.ap[0][0] == elem_step
    stride_bytes = elem_step * mybir.dt.size(in_ap.dtype)
    stride_bytes_256 = stride_bytes // 256
    assert stride_bytes % 256 == 0 and stride_bytes_256 < 256
    _in_ap = gp.lower_ap_dma(in_ap, for_custom_bir_dma=True)
    _idxs_ap = gp.lower_ap(idxs_ap)
    _out_ap = gp.lower_ap(out_ap)
    return gp.add_instruction(
        mybir.InstDMAGatherAnt(
            name=gp.bass.get_next_instruction_name(),
            ins=[*_in_ap, _idxs_ap, gp.lower_val_access(gp.to_reg(num_idxs))],
            outs=[_out_ap],
            transpose=False,
            num_idxs=num_idxs,
            elem_size=elem_size,
            stride_bytes_256=stride_bytes_256,
            gen_mode=0,
            single_packet=False,
            queue_num=queue_num,
            sbuf_tokens_per_rank=0,
            sbuf_free_dim_per_rank=0,
            sbuf_free_dim_pad_per_rank=0,
            sbuf_byte_offset=0,
        ))

F32 = mybir.dt.float32
F32R = mybir.dt.float32r
I16 = mybir.dt.int16

N0, IN = 1048576, 128
E0, ND0 = 2097152, 131072
E1, ND1 = 131072, 8192
HID, OUTF, PHEAD = 256, 256, 64
NCORES = 8
P = 128
NB1 = 128   # dst blocks per core, layer 1
NB2 = 8     # dst blocks per core, layer 2

TRACE = False  # test harness may flip this for profiling


# ----------------------------------------------------------------------------
# Host-side scheduling
# ----------------------------------------------------------------------------

def _schedule(dst_arr, nd, nb):
    """Assign each dst id to (core, block, slot); every block holds exactly P
    dsts with near-equal total degree (serpentine deal by degree)."""
    deg = np.bincount(dst_arr, minlength=nd)
    dorder = np.argsort(-deg, kind="stable")
    i = np.arange(nd)
    r, pos = divmod(i, NCORES)
    serp = np.where(r % 2 == 0, pos, NCORES - 1 - pos)
    core_of = np.empty(nd, np.int64)
    core_of[dorder] = serp

    block_of = np.empty(nd, np.int64)
    dstids_blocked = np.empty((NCORES, nb * P), np.int64)
    for c in range(NCORES):
        ids = dorder[core_of[dorder] == c]
        m = ids.size
        assert m == nb * P, (m, nb, P)
        j = np.arange(m)
        rb, pb = divmod(j, nb)
        serpb = np.where(rb % 2 == 0, pb, nb - 1 - pb)
        block_of[ids] = serpb
        ord2 = np.lexsort((rb, serpb))
        dstids_blocked[c] = ids[ord2]
    return core_of, block_of, dstids_blocked


def _edge_layout(src_arr, dst_arr, x_table, nd, nb):
    """Per-core device arrays for one layer.

    Returns dict with per-core lists:
      xtab [nb*RCAP, fin]  region tables (block b's distinct src rows at
                           rows [b*RCAP, ...))
      idxw [P, nb*kmax*8]  int16 wrapped region-local gather indices
      dstl [P, nb*kmax]    f32 dst slot within block (-1 dummy)
      xself [nb*P, fin]    self features in block order
    plus dstids_blocked and kmax.
    Edge slot i of block b maps to gather out (p=i%128, col=i//128); the
    matmul chunk j consumes slots {j*128+p}.
    """
    fin = x_table.shape[1]
    core_of, block_of, dstids_blocked = _schedule(dst_arr, nd, nb)
    slot_of = np.empty(nd, np.int64)
    for c in range(NCORES):
        slot_of[dstids_blocked[c]] = np.arange(nb * P) % P

    eco = core_of[dst_arr]
    ebl = block_of[dst_arr]
    esl = slot_of[dst_arr]

    # kmax: max edges in any (core, block)
    bsizes = np.bincount(eco * nb + ebl, minlength=NCORES * nb)
    kmax = int(np.ceil(bsizes.max() / P))
    rcap = kmax * P

    out = {"xtab": [], "idxw": [], "dstl": [], "xself": []}
    for c in range(NCORES):
        mask = eco == c
        s_e = src_arr[mask]
        b_e = ebl[mask]
        d_e = esl[mask]
        ne = s_e.size

        # slot assignment: rank within block (original order)
        border = np.argsort(b_e, kind="stable")
        bs = np.bincount(b_e, minlength=nb)
        starts = np.zeros(nb + 1, np.int64)
        np.cumsum(bs, out=starts[1:])
        rank = np.arange(ne) - starts[b_e[border]]
        # slot i = rank; gather layout (p = i % P, j = i // P)
        jj, pp = divmod(rank, P)

        # distinct (block, src) pairs -> region-local ids
        o2 = np.lexsort((s_e, b_e))
        k_b, k_s = b_e[o2], s_e[o2]
        first = np.ones(ne, bool)
        first[1:] = (k_b[1:] != k_b[:-1]) | (k_s[1:] != k_s[:-1])
        gid_sorted = np.cumsum(first) - 1
        dblk = k_b[first]
        dsrc = k_s[first]
        nd_ct = np.bincount(dblk, minlength=nb)
        dbase = np.zeros(nb + 1, np.int64)
        np.cumsum(nd_ct, out=dbase[1:])
        local_d = np.arange(dsrc.size) - dbase[dblk]
        assert local_d.max() < rcap
        # per-edge local idx (in o2 order), then scatter to slots
        edge_local_o2 = local_d[gid_sorted]
        edge_local = np.empty(ne, np.int64)
        edge_local[o2] = edge_local_o2

        seq = np.zeros((nb, rcap), np.int16)
        seq[b_e[border], jj * P + pp] = edge_local[border]
        dstl = np.full((nb, rcap), -1.0, np.float32)
        dstl[b_e[border], jj * P + pp] = d_e[border]

        # wrapped idx: per half-block segment [16, seg/16] replicated to 128
        kh = (kmax + 1) // 2
        segs = []
        for b in range(nb):
            for k0, k1 in ((0, kh), (kh, kmax)):
                seg = seq[b, k0 * P:k1 * P]
                segs.append(seg.reshape(-1, 16).T)  # [16, seg/16]
        idxw = np.tile(np.concatenate(segs, axis=1), (8, 1))

        # dstl in device layout [P, nb*kmax]: dstl_dev[p, b*kmax+j] = dstl[b, j*128+p]
        dstl_dev = dstl.reshape(nb, kmax, P).transpose(2, 0, 1).reshape(P, nb * kmax)

        rstride = (fin + 64 // 1) // 64 * 64 + 64  # fin+1 padded to 64-elem (256B) multiple
        rstride = ((fin + 1 + 63) // 64) * 64
        xtab = np.zeros((nb * rcap, rstride), np.float32)
        xtab[dblk * rcap + local_d, :fin] = x_table[dsrc]
        xtab[:, fin] = 1.0

        out["xtab"].append(xtab)
        out["idxw"].append(np.ascontiguousarray(idxw))
        out["dstl"].append(np.ascontiguousarray(dstl_dev))
        out["xself"].append(np.ascontiguousarray(x_table[dstids_blocked[c]]))
    return out, dstids_blocked, kmax


# ----------------------------------------------------------------------------
# Device program
# ----------------------------------------------------------------------------

def _build_sage_layer(nb, kmax, fin, fout, head=False):
    nc = bacc.Bacc("TRN2", target_bir_lowering=False, debug=False,
                   num_devices=NCORES, num_swdge_queues=4)
    rcap = kmax * P
    rstride = ((fin + 1 + 63) // 64) * 64
    xtab = nc.dram_tensor("xtab", [nb * rcap, rstride], F32R, kind="ExternalInput").ap()
    idx_t = nc.dram_tensor("idx", [P, nb * (rcap // 16)], I16, kind="ExternalInput").ap()
    dstl_t = nc.dram_tensor("dstl", [P, nb * kmax], F32, kind="ExternalInput").ap()
    xself_t = nc.dram_tensor("xself", [nb * P, fin], F32, kind="ExternalInput").ap()
    wT_t = nc.dram_tensor("wT", [fin, fout], F32R, kind="ExternalInput").ap()
    br_t = nc.dram_tensor("br", [P, fout // P], F32, kind="ExternalInput").ap()
    if head:
        whT_t = nc.dram_tensor("whT", [fout, PHEAD], F32R, kind="ExternalInput").ap()
        bhr_t = nc.dram_tensor("bhr", [PHEAD, 1], F32, kind="ExternalInput").ap()
        out_t = nc.dram_tensor("outT", [PHEAD, nb * P], F32, kind="ExternalOutput").ap()
    else:
        out_t = nc.dram_tensor("hT", [fout, nb * P], F32, kind="ExternalOutput").ap()

    foc = fout // P
    fic = fin // P
    gbufs = 10 if fin <= 128 else 5
    ohbufs = 6 if fin <= 128 else 4

    with tile.TileContext(nc) as tc:
        with tc.tile_pool(name="const", bufs=1) as cpool, \
             tc.tile_pool(name="idxp", bufs=1) as ixpool, \
             tc.tile_pool(name="g", bufs=gbufs) as gpool, \
             tc.tile_pool(name="oh", bufs=ohbufs) as ohpool, \
             tc.tile_pool(name="sb", bufs=3) as spool, \
             tc.tile_pool(name="pseg", bufs=2, space="PSUM") as pspool, \
             tc.tile_pool(name="ptp", bufs=2, space="PSUM") as tppool, \
             tc.tile_pool(name="pfc", bufs=2, space="PSUM") as fcpool:

            ident = cpool.tile([P, P], F32)
            make_identity(nc, ident[:])
            ones_t = cpool.tile([P, 1], F32, tag="ones")
            nc.vector.memset(ones_t[:], 1.0)
            wt_tiles = []
            for kc in range(fic):
                t = cpool.tile([P, fout], F32R, tag=f"wt{kc}")
                nc.sync.dma_start(t[:], wT_t[kc * P:(kc + 1) * P, :])
                wt_tiles.append(t)
            bt = cpool.tile([P, foc], F32, tag="bt")
            nc.sync.dma_start(bt[:], br_t)
            if head:
                wh_tiles = []
                for kc in range(foc):
                    t = cpool.tile([P, PHEAD], F32R, tag=f"wh{kc}")
                    nc.sync.dma_start(t[:], whT_t[kc * P:(kc + 1) * P, :])
                    wh_tiles.append(t)
                bh_tile = cpool.tile([PHEAD, 1], F32, tag="bh")
                nc.sync.dma_start(bh_tile[:], bhr_t)

            iotai = cpool.tile([P, P], mybir.dt.int32, tag="ioti")
            nc.gpsimd.iota(iotai[:], pattern=[[1, P]], channel_multiplier=0)
            iotaf = cpool.tile([P, P], F32, tag="iotf")
            nc.vector.tensor_copy(iotaf[:], iotai[:])
            iotaf_b = iotaf[:].rearrange("p (o d) -> p o d", o=1).to_broadcast(
                [P, kmax, P])

            idxt = ixpool.tile([P, nb * (rcap // 16)], I16, tag="idx")
            nc.sync.dma_start(idxt[:], idx_t)
            dslt = ixpool.tile([P, nb * kmax], F32, tag="dsl")
            nc.sync.dma_start(dslt[:], dstl_t)

            kh = (kmax + 1) // 2  # chunks in first half-call
            for b in range(nb):
                g = gpool.tile([P, kmax, fin + 2], F32R, tag="g")
                for h, (k0, k1) in enumerate(((0, kh), (kh, kmax))):
                    nidx = (k1 - k0) * P
                    _dma_gather_raw(
                        nc.gpsimd,
                        out_ap=g[:, k0:k1, :],
                        in_ap=xtab[b * rcap:(b + 1) * rcap, :],
                        idxs_ap=idxt[:, (b * kmax + k0) * 8:(b * kmax + k1) * 8],
                        num_idxs=nidx,
                        elem_size=fin + 2,
                        elem_step=rstride,
                        queue_num=(2 * b + h) % 4,
                    )

                oh = ohpool.tile([P, kmax, P], F32R, tag="oh")
                dsl_b = dslt[:, b * kmax:(b + 1) * kmax].rearrange(
                    "p (k o) -> p k o", o=1).to_broadcast([P, kmax, P])
                nc.vector.tensor_tensor(out=oh[:], in0=dsl_b, in1=iotaf_b,
                                        op=mybir.AluOpType.is_equal)

                ps = pspool.tile([P, fin + 2], F32, tag="pseg")
                for j in range(kmax):
                    nc.tensor.matmul(out=ps[:], lhsT=oh[:, j, :],
                                     rhs=g[:, j, :],
                                     start=(j == 0), stop=(j == kmax - 1))

                xs = spool.tile([P, fin], F32, tag="xs")
                nc.sync.dma_start(xs[:], xself_t[b * P:(b + 1) * P, :])
                s = spool.tile([P, fin], F32, tag="s")
                nc.vector.tensor_add(s[:], ps[:, 0:fin], xs[:])
                d1 = spool.tile([P, 1], F32, tag="d1")
                nc.vector.tensor_add(d1[:], ps[:, fin:fin + 1], ones_t[:])
                rc = spool.tile([P, 1], F32, tag="rc")
                nc.vector.reciprocal(rc[:], d1[:])
                hn = spool.tile([P, fin], F32, tag="hn")
                rc_b = rc[:].to_broadcast([P, fin])
                nc.vector.tensor_tensor(out=hn[:], in0=s[:], in1=rc_b,
                                        op=mybir.AluOpType.mult)

                hnT_tiles = []
                for kc in range(fic):
                    tp = tppool.tile([P, P], F32, tag="tp")
                    nc.tensor.transpose(tp[:], hn[:, kc * P:(kc + 1) * P], ident[:])
                    ht = spool.tile([P, P], F32R, tag=f"hnT{kc}")
                    nc.vector.tensor_copy(ht[:], tp[:])
                    hnT_tiles.append(ht)

                h_tiles = []
                for oc in range(foc):
                    pf = fcpool.tile([P, P], F32, tag="pfc")
                    for kc in range(fic):
                        nc.tensor.matmul(
                            out=pf[:],
                            lhsT=wt_tiles[kc][:, oc * P:(oc + 1) * P],
                            rhs=hnT_tiles[kc][:],
                            start=(kc == 0), stop=(kc == fic - 1))
                    hs = spool.tile([P, P], F32R, tag=f"hs{oc}")
                    nc.scalar.activation(hs[:], pf[:],
                                         mybir.ActivationFunctionType.Relu,
                                         bias=bt[:, oc:oc + 1], scale=1.0)
                    h_tiles.append(hs)
                    if not head:
                        nc.sync.dma_start(
                            out_t[oc * P:(oc + 1) * P, b * P:(b + 1) * P],
                            hs[:].bitcast(F32))

                if head:
                    ph = fcpool.tile([PHEAD, P], F32, tag="pfc")
                    for kc in range(foc):
                        nc.tensor.matmul(out=ph[:],
                                         lhsT=wh_tiles[kc][:],
                                         rhs=h_tiles[kc][:],
                                         start=(kc == 0), stop=(kc == foc - 1))
                    os_ = spool.tile([PHEAD, P], F32, tag="os")
                    nc.vector.tensor_scalar_add(os_[:], ph[:], bh_tile[:, 0:1])
                    nc.sync.dma_start(out_t[:, b * P:(b + 1) * P], os_[:])

    nc.compile()
    return nc


# ----------------------------------------------------------------------------
# Host orchestration
# ----------------------------------------------------------------------------

def _run_layer(feat_table, src_arr, dst_arr, nd, nb, w, bvec, head_w=None,
               head_b=None, debug=None, tag=""):
    fin = feat_table.shape[1]
    fout = w.shape[0]
    arrs, dstids_blocked, kmax = _edge_layout(src_arr, dst_arr, feat_table, nd, nb)

    wT = np.ascontiguousarray(w.T).astype(np.float32)
    br = np.ascontiguousarray(bvec.reshape(fout // P, P).T)

    in_maps = []
    for c in range(NCORES):
        m = {
            "xtab": arrs["xtab"][c],
            "idx": arrs["idxw"][c],
            "dstl": arrs["dstl"][c],
            "xself": arrs["xself"][c],
            "wT": wT,
            "br": br,
        }
        if head_w is not None:
            m["whT"] = np.ascontiguousarray(head_w.T).astype(np.float32)
            m["bhr"] = np.ascontiguousarray(head_b.reshape(PHEAD, 1)).astype(np.float32)
        in_maps.append(m)

    nc = _build_sage_layer(nb, kmax, fin, fout, head=head_w is not None)
    res = bass_utils.run_bass_kernel_spmd(
        nc, in_maps, core_ids=list(range(NCORES)), trace=TRACE)
    if debug is not None:
        debug.setdefault("exec_ns", {})[tag] = res.exec_time_ns
        debug.setdefault("trace", {})[tag] = (
            None if res.instructions_and_trace is None
            else res.instructions_and_trace[1])

    outname = "outT" if head_w is not None else "hT"
    outdim = PHEAD if head_w is not None else fout
    full = np.empty((nd, outdim), np.float32)
    for c in range(NCORES):
        full[dstids_blocked[c]] = res.results[c][outname].T
    return full


def kernel(x, src0, dst0, src1, dst1, W1, b1, W2, b2, Wh, bh,
           n_dst0, n_dst1, task_index, _debug=None):
    x = np.asarray(x, np.float32)
    src0 = np.asarray(src0).astype(np.int64)
    dst0 = np.asarray(dst0).astype(np.int64)
    src1 = np.asarray(src1).astype(np.int64)
    dst1 = np.asarray(dst1).astype(np.int64)
    W1 = np.asarray(W1, np.float32); b1 = np.asarray(b1, np.float32)
    W2 = np.asarray(W2, np.float32); b2 = np.asarray(b2, np.float32)
    Wh = np.asarray(Wh, np.float32); bh = np.asarray(bh, np.float32)

    h1 = _run_layer(x, src0, dst0, ND0, NB1, W1, b1, debug=_debug, tag="l1")
    out = _run_layer(h1, src1, dst1, ND1, NB2, W2, b2,
                     head_w=Wh, head_b=bh, debug=_debug, tag="l2")
    return out
